# revision 1
# baseline (speedup 1.0000x reference)
"""Neural ODE (RK4, tanh-MLP vector field) Trainium2 kernel.

Data-parallel over 8 NeuronCores: batch 8192 -> 1024/core, processed as
2 interleaved tiles of 512 (batch streamed as matmul free dim, features
on partitions). All matmuls run in fp32r (full-rate fp32, ~11-bit
mantissa inputs, fp32 accumulate); the integration state `cur` is kept
in full fp32 and only rounded copies feed the matmuls, so rounding
error does not compound across the 999 steps.

Per RK4 stage j (per tile):
  a1 = W1aug^T @ [s;1]        2 fp32r MMs (K=4; b1 and the c_j*dt*W1^T b3
                              bias terms folded into weight row 3)
  h1 = tanh(a1)               1 ACT op [128,1024] PSUM->SBUF
  a2 = W2^T @ h1              4 fp32r MMs (K=128 chunks)
  h2 = tanh(a2)               1 ACT op
  ktil = (c_j*dt*W3)^T @ h2   2 fp32r MMs -> PSUM   (j<4)
  nxt += (w_j*dt*W3)^T @ h2   2 fp32r MMs, PSUM-accumulated over 4 stages
  s_next = cur + ktil         DVE tensor add [3,512] (f32r out)
Step end: cur[dst] = cur[src] + nxt (DVE, ping-pong buffers), then an
SBUF->SBUF DMA writes the new state into a [111,512] staging buffer
(37 steps x 3 dims on partitions). Every 37 steps: 4 PE transposes
[111,128]->[128,111] + contiguous DMAs to the [B, T*3] output. Outer
For_i over 27 blocks; RK4 coefficients (mean fp32 dt) and biases are
folded into host-precomputed weights.
"""

import numpy as np

import concourse.bass as bass
import concourse.mybir as mybir
import concourse.tile as tile
from concourse import bacc
from concourse.bass_utils import run_bass_kernel_spmd
from concourse.masks import make_identity

F32 = mybir.dt.float32
F32R = mybir.dt.float32r
TANH = mybir.ActivationFunctionType.Tanh

B = 8192          # total batch
T = 1000          # total states (999 steps)
D = 3             # state dim
H = 256           # hidden dim
NCORES = 8
BS = B // NCORES  # 1024 batch per core
NT = 2            # batch tiles per core
NB = BS // NT     # 512 batch per tile (fp32 moving-operand max)
TBLK = 37         # steps per staging block (37*3=111 partitions)


def _r(ap):
    return ap.bitcast(F32R)


def build_nc(t_total=T, has_b2=False, has_b3=False, reps=1, nodve=False):
    """Build+compile the Bass module. t_total-1 must be divisible by TBLK."""
    nsteps = t_total - 1
    assert nsteps % TBLK == 0
    nblk = nsteps // TBLK

    nc = bacc.Bacc("TRN2", target_bir_lowering=False, debug=False)

    init_d = nc.dram_tensor("init_t", [NT, D, NB], F32, kind="ExternalInput")
    w1a_d = nc.dram_tensor("w1a", [4, 6, 128], F32, kind="ExternalInput")
    w2h_d = nc.dram_tensor("w2h", [128, 4, 128], F32, kind="ExternalInput")
    w3s_d = nc.dram_tensor("w3s", [128, 14, D], F32, kind="ExternalInput")
    b2h_d = nc.dram_tensor("b2h", [128, 2], F32, kind="ExternalInput")
    b3f_d = nc.dram_tensor("b3f", [D, 1], F32, kind="ExternalInput")
    roll_d = nc.dram_tensor("roll", [BS, t_total * D], F32, kind="ExternalOutput")

    with tile.TileContext(nc) as tc:
        with (
            tc.tile_pool(name="const", bufs=1) as constp,
            tc.tile_pool(name="state", bufs=1) as statep,
            tc.tile_pool(name="hbuf", bufs=2) as hbuf,
            tc.tile_pool(name="psA", bufs=1, space="PSUM") as psA,
            tc.tile_pool(name="psK", bufs=1, space="PSUM") as psK,
        ):
            # ---- constants ----
            w1sb = constp.tile([4, 6 * 128], F32R, tag="w1sb")
            nc.sync.dma_start(out=w1sb, in_=w1a_d[:, :, :].bitcast(F32R))
            w2sb = constp.tile([128, 4 * 128], F32R, tag="w2sb")
            nc.sync.dma_start(out=w2sb, in_=w2h_d[:, :, :].bitcast(F32R))
            w3sb = constp.tile([128, 14 * D], F32R, tag="w3sb")
            nc.sync.dma_start(out=w3sb, in_=w3s_d[:, :, :].bitcast(F32R))
            b2sb = constp.tile([128, 2], F32, tag="b2sb")
            nc.sync.dma_start(out=b2sb, in_=b2h_d[:, :])
            b3sb = constp.tile([D, 1], F32, tag="b3sb")
            nc.sync.dma_start(out=b3sb, in_=b3f_d[:, :])
            ident = constp.tile([128, 128], F32, tag="ident")
            make_identity(nc, ident)

            # ---- persistent state ----
            cur = [[statep.tile([4, NB], F32, tag=f"cur{t}_{p}", name=f"cur{t}_{p}")
                    for p in range(2)] for t in range(NT)]
            stmp = [statep.tile([4, NB], F32R, tag=f"stmp{t}", name=f"stmp{t}") for t in range(NT)]
            cur_r = [statep.tile([4, NB], F32R, tag=f"curr{t}", name=f"curr{t}") for t in range(NT)]
            stag = [statep.tile([TBLK * D, NB], F32, tag=f"stag{t}", name=f"stag{t}") for t in range(NT)]
            for t in range(NT):
                nc.vector.memset(cur[t][0][0:4, :], 1.0)
                nc.vector.memset(cur[t][1][0:4, :], 1.0)
                nc.vector.memset(stmp[t][0:4, :].bitcast(F32), 1.0)
                nc.sync.dma_start(out=cur[t][0][0:3, :], in_=init_d[t, :, :])
                nc.sync.dma_start(out=stag[t][0:3, :], in_=init_d[t, :, :])
                if nodve:
                    nc.vector.tensor_copy(cur_r[t][0:4, :], cur[t][0][0:4, :])

            # lhsT slices
            def w1_lhsT(v, c):  # bias variant v (0..2), m-chunk c
                return w1sb[:, (v * 2 + c) * 128:(v * 2 + c + 1) * 128]

            def w2_lhsT(kc, mc):
                return w2sb[:, (kc * 2 + mc) * 128:(kc * 2 + mc + 1) * 128]

            def w3_lhsT(j, kc, is_kx):  # 0..5 ktil (j*2+kc), 6..13 nxt
                i = (j * 2 + kc) if is_kx else (6 + j * 2 + kc)
                return w3sb[:, i * D:(i + 1) * D]

            # bias variant per stage: stage0 -> v0, stage1/2 -> v1, stage3 -> v2
            STAGE_V = (0, 1, 1, 2)

            def one_step(sp, dp):
                """One RK4 step for both tiles; reads cur[.][sp], writes cur[.][dp]."""
                nxt = [psK.tile([D, NB], F32, tag="nx", name=f"nx{t}", bufs=2)
                       for t in range(NT)]
                for t in range(NT):
                    if not nodve:
                        nc.vector.tensor_copy(cur_r[t][0:4, :], cur[t][sp][0:4, :])
                for j in range(4):
                    a1 = {}
                    h1 = {}
                    a2 = {}
                    h2 = {}
                    knx = {}
                    for t in range(NT):
                        s_in = cur_r[t] if j == 0 else stmp[t]
                        a1[t] = psA.tile([128, 2 * NB], F32, tag="aa", name=f"aa{t}", bufs=2)
                        for c in range(2):
                            nc.tensor.matmul(
                                a1[t][:, c * NB:(c + 1) * NB],
                                w1_lhsT(STAGE_V[j], c),
                                s_in[0:4, :],
                                start=True, stop=True,
                            )
                    for t in range(NT):
                        h1[t] = hbuf.tile([128, 2 * NB], F32R, tag=f"h1_{t}", name=f"h1_{t}")
                        nc.scalar.activation(h1[t], a1[t], TANH)
                    for t in range(NT):
                        a2[t] = psA.tile([128, 2 * NB], F32, tag="aa", name=f"aa{t}", bufs=2)
                        for mc in range(2):
                            for kc in range(2):
                                nc.tensor.matmul(
                                    a2[t][:, mc * NB:(mc + 1) * NB],
                                    w2_lhsT(kc, mc),
                                    h1[t][:, kc * NB:(kc + 1) * NB],
                                    start=(kc == 0), stop=(kc == 1),
                                )
                    for t in range(NT):
                        h2[t] = hbuf.tile([128, 2 * NB], F32R, tag=f"h2_{t}", name=f"h2_{t}")
                        if has_b2:
                            for mc in range(2):
                                nc.scalar.activation(
                                    h2[t][:, mc * NB:(mc + 1) * NB],
                                    a2[t][:, mc * NB:(mc + 1) * NB],
                                    TANH, bias=b2sb[:, mc:mc + 1],
                                )
                        else:
                            nc.scalar.activation(h2[t], a2[t], TANH)
                    for t in range(NT):
                        if j < 3:
                            knx[t] = psK.tile([D, NB], F32, tag="kx", name=f"kx{t}", bufs=2)
                            for kc in range(2):
                                nc.tensor.matmul(
                                    knx[t][0:D, :],
                                    w3_lhsT(j, kc, True),
                                    h2[t][:, kc * NB:(kc + 1) * NB],
                                    start=(kc == 0), stop=(kc == 1),
                                )
                        for kc in range(2):
                            nc.tensor.matmul(
                                nxt[t][0:D, :],
                                w3_lhsT(j, kc, False),
                                h2[t][:, kc * NB:(kc + 1) * NB],
                                start=(j == 0 and kc == 0),
                                stop=(j == 3 and kc == 1),
                            )
                    for t in range(NT):
                        if nodve or j >= 3:
                            continue
                        nc.vector.tensor_add(
                            stmp[t][0:3, :], cur[t][sp][0:3, :], knx[t][0:D, :]
                        )
                for t in range(NT):
                    if nodve:
                        continue
                    nc.vector.tensor_add(
                        cur[t][dp][0:3, :], cur[t][sp][0:3, :], nxt[t][0:D, :]
                    )
                    if has_b3:
                        nc.vector.tensor_scalar(
                            cur[t][dp][0:3, :], cur[t][dp][0:3, :],
                            b3sb[0:3, :], None, mybir.AluOpType.add,
                        )

            def stage_write(t, tb, p):
                nc.sync.dma_start(
                    out=stag[t][3 * tb:3 * tb + 3, :], in_=cur[t][p][0:3, :]
                )

            def flush(iv, nslots):
                """Transpose staging and DMA to DRAM. iv = block idx expr."""
                ncols = nslots * D
                for t in range(NT):
                    for c in range(4):
                        trn = psK.tile([128, TBLK * D], F32, tag="kx", name=f"trn{t}", bufs=2)
                        nc.tensor.transpose(
                            trn[0:128, 0:ncols],
                            stag[t][0:ncols, c * 128:(c + 1) * 128],
                            ident[0:ncols, 0:ncols],
                        )
                        fo = hbuf.tile([128, TBLK * D], F32, tag=f"fo{t}", name=f"fo{t}")
                        nc.vector.tensor_copy(fo[:, 0:ncols], trn[0:128, 0:ncols])
                        nc.sync.dma_start(
                            out=roll_d[
                                t * NB + c * 128: t * NB + (c + 1) * 128,
                                bass.ds(iv * (TBLK * D), ncols),
                            ],
                            in_=fo[:, 0:ncols],
                        )

            # ---- main loop over blocks ----
            with tc.For_i(0, nblk, hint_engines=tuple(mybir.ALL_ENGINES)) as iv:
                for rep in range(reps):
                    for i in range(TBLK - 1):
                        sp, dp = i % 2, (i + 1) % 2
                        one_step(sp, dp)
                        for t in range(NT):
                            stage_write(t, i + 1, dp)
                    if rep == reps - 1:
                        flush(iv, TBLK)
                    # last step: back into buffer 0 (in-place read is buf 0 iff TBLK odd)
                    one_step((TBLK - 1) % 2, 0)
                    for t in range(NT):
                        stage_write(t, 0, 0)

            # state t_total-1 sits in staging slot 0
            for t in range(NT):
                for c in range(4):
                    trn = psK.tile([128, TBLK * D], F32, tag="kx", name=f"trn{t}", bufs=2)
                    nc.tensor.transpose(
                        trn[0:128, 0:D],
                        stag[t][0:D, c * 128:(c + 1) * 128],
                        ident[0:D, 0:D],
                    )
                    fo = hbuf.tile([128, TBLK * D], F32, tag=f"fo{t}", name=f"fo{t}")
                    nc.vector.tensor_copy(fo[:, 0:D], trn[0:128, 0:D])
                    nc.sync.dma_start(
                        out=roll_d[
                            t * NB + c * 128: t * NB + (c + 1) * 128,
                            (t_total - 1) * D: t_total * D,
                        ],
                        in_=fo[:, 0:D],
                    )

    nc.compile()
    return nc


_NC_CACHE = {}


def _get_nc(t_total, has_b2, has_b3, reps=1, nodve=False):
    key = (t_total, has_b2, has_b3, reps, nodve)
    if key not in _NC_CACHE:
        _NC_CACHE[key] = build_nc(t_total, has_b2, has_b3, reps, nodve)
    return _NC_CACHE[key]


def _prep_inputs(initial_state, t_grid, W1, b1, W2, b2, W3, b3, t_total):
    """Host-side packing of weights with RK4 coefficients folded in."""
    dts = np.diff(np.asarray(t_grid, np.float64))
    dt = float(dts.mean())
    W1_64 = np.asarray(W1, np.float64)
    W3_64 = np.asarray(W3, np.float64)
    b1_64 = np.asarray(b1, np.float64)
    b3_64 = np.asarray(b3, np.float64)

    # w1a: [4, 6, 128] = (k+bias row, variant*chunk, m)
    w1t_b3 = W1_64.T @ b3_64  # [256]
    w1a = np.zeros((4, 6, 128), np.float64)
    for v, cv in enumerate((0.0, 0.5, 1.0)):
        bias_v = b1_64 + cv * dt * w1t_b3
        for c in range(2):
            w1a[0:3, v * 2 + c, :] = W1_64[:, c * 128:(c + 1) * 128]
            w1a[3, v * 2 + c, :] = bias_v[c * 128:(c + 1) * 128]

    # w2h: [128, (kc*2+mc), 128]
    w2h = (
        np.asarray(W2, np.float64)
        .reshape(2, 128, 2, 128)
        .transpose(1, 0, 2, 3)
        .reshape(128, 4, 128)
    )

    # w3s: [128, 14, D]: 0..5 ktil (j in 0..2, kc), 6..13 nxt (j in 0..3, kc)
    kt_scales = (0.5 * dt, 0.5 * dt, dt)
    nx_scales = (dt / 6, dt / 3, dt / 3, dt / 6)
    w3s = np.zeros((128, 14, D), np.float64)
    for j, s in enumerate(kt_scales):
        sw = (W3_64 * s).reshape(2, 128, D)
        for kc in range(2):
            w3s[:, j * 2 + kc, :] = sw[kc]
    for j, s in enumerate(nx_scales):
        sw = (W3_64 * s).reshape(2, 128, D)
        for kc in range(2):
            w3s[:, 6 + j * 2 + kc, :] = sw[kc]

    b2h = np.asarray(b2, np.float64).reshape(2, 128).T  # [128, 2]
    b3f = (dt * b3_64).reshape(D, 1)

    shared = {
        "w1a": w1a.astype(np.float32),
        "w2h": w2h.astype(np.float32),
        "w3s": w3s.astype(np.float32),
        "b2h": np.ascontiguousarray(b2h.astype(np.float32)),
        "b3f": b3f.astype(np.float32),
    }

    init = np.asarray(initial_state, np.float32)  # [B, 3]
    in_maps = []
    for core in range(NCORES):
        shard = init[core * BS:(core + 1) * BS]  # [BS, 3]
        init_t = (
            shard.reshape(NT, NB, D).transpose(0, 2, 1).copy()
        )  # [NT, D, NB]
        in_maps.append({"init_t": init_t, **shared})
    return in_maps


def _run(initial_state, t_grid, W1, b1, W2, b2, W3, b3, t_total=T, reps=1, **run_kwargs):
    has_b2 = bool(np.any(np.asarray(b2) != 0))
    has_b3 = bool(np.any(np.asarray(b3) != 0))
    nc = _get_nc(t_total, has_b2, has_b3, reps)
    in_maps = _prep_inputs(
        initial_state, t_grid, W1, b1, W2, b2, W3, b3, t_total
    )
    res = run_bass_kernel_spmd(nc, in_maps, core_ids=list(range(NCORES)), **run_kwargs)
    roll = np.concatenate(
        [res.results[c]["roll"].reshape(BS, t_total, D) for c in range(NCORES)],
        axis=0,
    )
    roll[:, 0, :] = np.asarray(initial_state, np.float32)
    return roll, res


def kernel(initial_state, t_grid, W1, b1, W2, b2, W3, b3):
    roll, _ = _run(initial_state, t_grid, W1, b1, W2, b2, W3, b3)
    return roll

